# revision 17
# baseline (speedup 1.0000x reference)
"""Trainium2 Bass kernel for EruSelfAttentionModel.

Math (reference, simplified):
  e  = emb_table[x]                                  # [B,S,E] gather
  h  = LayerNorm(e) * gamma + beta                   # over E
  q  = einsum('hae,bse->bhsa', Wq, h); k likewise    # A=64 per head
  v  = einsum('hve,bse->bhsv', Wv, h)                # v-dim = E
  scores = q @ k^T / sqrt(E)
  sn = (scores - min) / (max - min)  (rowwise)
  softmax_sel = 1 - max(sn) == 0 exactly  =>  weights = sigmoid(10*sn - 5)
  out = weights @ v                                  # [B,H,S,E]

Key identities used:
  - sn is invariant to positive rescaling of scores => the 1/sqrt(E) scale
    can be dropped entirely.
  - weights = sigmoid(alpha * scores + beta_row) with per-row
    alpha = 10/(mx-mn), beta_row = -10*mn/(mx-mn) - 5  => single fused
    ScalarE activation pass (per-partition scale/bias APs).

Sharding: data-parallel over batch; core b computes batch b fully.
"""

import os
import sys

sys.path.insert(0, "/opt/trn_rl_repo")

import numpy as np
import ml_dtypes

import concourse.bass as bass
import concourse.bacc as bacc
import concourse.tile as tile
from concourse import mybir
from concourse.bass_utils import run_bass_kernel_spmd
import concourse.bass_utils as _bass_utils

# Let walrus overlap LDWEIGHTS with in-flight matmuls (the PE pulls LDW for
# the background weight buffer ahead); without it every matmul serializes
# behind its weight load (~+160ns per matmul).
if not getattr(_bass_utils, "_ldw_patched", False):
    _orig_run_command = _bass_utils.run_command

    def _patched_run_command(argv, **kwargs):
        if os.environ.get("KERNEL_LDW_OPT", "1") == "1":
            argv = [
                a.replace("--enable-ldw-opt=false", "--enable-ldw-opt=true")
                if isinstance(a, str) else a
                for a in argv
            ]
        return _orig_run_command(argv, **kwargs)

    _bass_utils.run_command = _patched_run_command
    _bass_utils._ldw_patched = True

BF16 = ml_dtypes.bfloat16

VOCAB, E, A, H = 32000, 512, 64, 8
B, S = 8, 1024
P = 128                 # partitions
NCH = S // P            # 8 token chunks
EC = E // P             # 4 embedding chunks
LN_EPS = 1e-5

F32 = mybir.dt.float32
BF = mybir.dt.bfloat16
I16 = mybir.dt.int16

_BUILD_CACHE = {}
LAST_RESULTS = None     # test.py reads exec_time_ns from here


def build_nc(use_beta: bool, debug: bool = False):
    if (use_beta, debug) in _BUILD_CACHE:
        return _BUILD_CACHE[(use_beta, debug)]

    nc = bacc.Bacc("TRN2", target_bir_lowering=False, num_devices=8)

    idx_d = nc.declare_dram_parameter("idx", [P, S // 16], I16, isOutput=False)
    emb_d = nc.declare_dram_parameter("emb", [VOCAB, E], F32, isOutput=False)
    wqt_d = nc.declare_dram_parameter("wqt", [E, H * A], BF, isOutput=False)
    wkt_d = nc.declare_dram_parameter("wkt", [E, H * A], BF, isOutput=False)
    wvt_d = nc.declare_dram_parameter("wvt", [E, H * E], BF, isOutput=False)
    idn_d = nc.declare_dram_parameter("idn", [P, P], BF, isOutput=False)
    if use_beta:
        qb_d = nc.declare_dram_parameter("qb", [P, 4], F32, isOutput=False)
        kb_d = nc.declare_dram_parameter("kb", [P, 4], F32, isOutput=False)
        vb_d = nc.declare_dram_parameter("vb", [1, H * E], F32, isOutput=False)
    out_d = nc.declare_dram_parameter("out", [H, S, E], BF, isOutput=True)
    if debug:
        dbg_ht_d = nc.declare_dram_parameter("dbg_ht", [P, EC, S], BF, isOutput=True)
        dbg_qt_d = nc.declare_dram_parameter("dbg_qt", [P, EC, S], BF, isOutput=True)
        dbg_kt_d = nc.declare_dram_parameter("dbg_kt", [P, EC, S], BF, isOutput=True)
        dbg_vh_d = nc.declare_dram_parameter("dbg_vh", [P, NCH, H * E], BF, isOutput=True)
        dbg_w_d = nc.declare_dram_parameter("dbg_w", [P, NCH, P], BF, isOutput=True)
        dbg_wp_d = nc.declare_dram_parameter("dbg_wp", [P, S], BF, isOutput=True)
        dbg_sc_d = nc.declare_dram_parameter("dbg_sc", [P, S], F32, isOutput=True)
        dbg_st_d = nc.declare_dram_parameter("dbg_st", [P, 8], F32, isOutput=True)

    with tile.TileContext(nc) as tc:
        with tc.tile_pool(name="consts", bufs=1) as consts:
            idx_sb = consts.tile([P, S // 16], I16)
            nc.sync.dma_start(idx_sb[:], idx_d[:])
            idn_sb = consts.tile([P, P], BF)
            nc.sync.dma_start(idn_sb[:], idn_d[:])
            wqt_sb = consts.tile([P, EC, H * A], BF)
            nc.sync.dma_start(
                wqt_sb[:], wqt_d.ap().rearrange("(ec p) j -> p ec j", p=P)
            )
            wkt_sb = consts.tile([P, EC, H * A], BF)
            nc.sync.dma_start(
                wkt_sb[:], wkt_d.ap().rearrange("(ec p) j -> p ec j", p=P)
            )
            wvt_sb = consts.tile([P, EC, H * E], BF)
            nc.sync.dma_start(
                wvt_sb[:], wvt_d.ap().rearrange("(ec p) j -> p ec j", p=P)
            )
            eps_sb = consts.tile([P, 1], F32)
            nc.vector.memset(eps_sb[:], LN_EPS)

            if use_beta:
                qb_sb = consts.tile([P, 4], F32)
                nc.sync.dma_start(qb_sb[:], qb_d[:])
                kb_sb = consts.tile([P, 4], F32)
                nc.sync.dma_start(kb_sb[:], kb_d[:])
                vb_sb = consts.tile([P, H * E], F32)
                vb_bcast = bass.AP(
                    tensor=vb_d, offset=0, ap=[[0, P], [1, H * E]]
                )
                nc.sync.dma_start(vb_sb[:], vb_bcast)

            # persistent activations
            hT_sb = consts.tile([P, EC, S], BF)       # hT[e%128, e//128, s]
            qT_sb = consts.tile([P, EC, S], BF)       # qT[ha%128, ha//128, s]
            kT_sb = consts.tile([P, EC, S], BF)
            vh_sb = consts.tile([P, NCH, H * E], BF)  # vh[p, c, v] = V[8p+c, v]

            # ---------------- phase A: gather + LN + transpose ----------------
            with (
                tc.tile_pool(name="e_pool", bufs=1) as e_pool,
                tc.tile_pool(name="h_pool", bufs=3) as h_pool,
                tc.tile_pool(name="st_pool", bufs=8) as st_pool,
                tc.tile_pool(name="pt_psum", bufs=4, space="PSUM") as pt_psum,
            ):
                e_ts = []
                mv = st_pool.tile([P, NCH, 2], F32, tag="mv")
                for c in range(NCH):
                    e_t = e_pool.tile([P, 1, E], F32, tag=f"e{c}")
                    nc.gpsimd.dma_gather(
                        e_t[:], emb_d.ap(), idx_sb[:, 8 * c : 8 * (c + 1)],
                        P, P, E,
                    )
                    e_ts.append(e_t)
                    stt = st_pool.tile([P, 6], F32, tag="bn")
                    nc.vector.bn_stats(stt[:], e_t[:, 0, :])
                    nc.vector.bn_aggr(mv[:, c, :], stt[:])
                # batched inv-std: var <- 1/sqrt(var+eps) for all 8 chunks
                var_ap = mv[:, :, 1]
                nc.scalar.activation(
                    out=var_ap, in_=var_ap,
                    func=mybir.ActivationFunctionType.Sqrt,
                    bias=eps_sb[:, 0:1], scale=1.0,
                )
                nc.vector.reciprocal(var_ap, var_ap)
                for c in range(NCH):
                    h_t = h_pool.tile([P, E], BF)
                    nc.vector.tensor_scalar(
                        out=h_t[:], in0=e_ts[c][:, 0, :],
                        scalar1=mv[:, c, 0:1], scalar2=mv[:, c, 1:2],
                        op0=mybir.AluOpType.subtract, op1=mybir.AluOpType.mult,
                    )
                    for ec in range(EC):
                        pt = pt_psum.tile([P, P], BF)
                        nc.tensor.transpose(
                            pt[:], h_t[:, ec * P : (ec + 1) * P], idn_sb[:]
                        )
                        nc.any.tensor_copy(
                            hT_sb[:, ec, c * P : (c + 1) * P], pt[:]
                        )

            # ---------------- phase B: projections ----------------
            with tc.tile_pool(name="proj_psum", bufs=4, space="PSUM") as proj_psum:
                for c in range(NCH):  # t-groups: partition p holds t = c*128+p
                    for vp in range(4):  # 1024-wide v pairs
                        pv = proj_psum.tile([P, S], F32, tag="pp")
                        for ec in range(EC):
                            lhsT = hT_sb[:, ec, c * P : (c + 1) * P]
                            for nn in range(2):
                                lo = vp * 1024 + nn * 512
                                nc.tensor.matmul(
                                    pv[:, nn * 512 : (nn + 1) * 512],
                                    lhsT,
                                    wvt_sb[:, ec, lo : lo + 512],
                                    start=(ec == 0), stop=(ec == EC - 1),
                                )
                        if use_beta:
                            nc.vector.tensor_add(
                                out=pv[:], in0=pv[:],
                                in1=vb_sb[:, vp * 1024 : (vp + 1) * 1024],
                            )
                        nc.any.tensor_copy(
                            vh_sb[:, c, vp * 1024 : (vp + 1) * 1024], pv[:]
                        )

                for name, w_sb, t_sb, b_sb in (
                    ("q", wqt_sb, qT_sb, "qb"),
                    ("k", wkt_sb, kT_sb, "kb"),
                ):
                    for sl in range(4):  # 128-wide (h,a) slices = head pairs
                        pq = proj_psum.tile([P, S], F32, tag="pp")
                        for ec in range(EC):
                            lhsT = w_sb[:, ec, sl * P : (sl + 1) * P]
                            for nn in range(2):
                                nc.tensor.matmul(
                                    pq[:, nn * 512 : (nn + 1) * 512],
                                    lhsT,
                                    hT_sb[:, ec, nn * 512 : (nn + 1) * 512],
                                    start=(ec == 0), stop=(ec == EC - 1),
                                )
                        if use_beta:
                            bb = qb_sb if name == "q" else kb_sb
                            nc.vector.tensor_scalar_add(
                                out=pq[:], in0=pq[:], scalar1=bb[:, sl : sl + 1]
                            )
                        nc.any.tensor_copy(t_sb[:, sl, :], pq[:])

            if debug:
                nc.sync.dma_start(dbg_ht_d[:], hT_sb[:])
                nc.sync.dma_start(dbg_qt_d[:], qT_sb[:])
                nc.sync.dma_start(dbg_kt_d[:], kT_sb[:])
                nc.sync.dma_start(dbg_vh_d[:], vh_sb[:])

            # ---------------- phase C: attention ----------------
            with (
                tc.tile_pool(name="sc_psum", bufs=3, space="PSUM") as sc_psum,
                tc.tile_pool(name="out_psum", bufs=2, space="PSUM") as out_psum,
                tc.tile_pool(name="sstat", bufs=8) as sstat,
                tc.tile_pool(name="w_pool", bufs=6) as w_pool,
                tc.tile_pool(name="wt_pool", bufs=6) as wt_pool,
                tc.tile_pool(name="ob_pool", bufs=4) as ob_pool,
            ):
                def out_stage(args):
                    hp_, i_, wts = args
                    for sub in range(2):
                        h_idx = 2 * hp_ + sub
                        wt_t = wts[sub]
                        po = out_psum.tile([P, E], F32, tag="po")
                        for cc in range(NCH):
                            nc.tensor.matmul(
                                po[:],
                                wt_t[:, cc, :],
                                vh_sb[:, cc, h_idx * E : (h_idx + 1) * E],
                                start=(cc == 0), stop=(cc == NCH - 1),
                            )
                        ob = ob_pool.tile([P, E], BF, tag="ob")
                        nc.scalar.copy(ob[:], po[:])
                        nc.sync.dma_start(
                            out_d[h_idx, i_ * P : (i_ + 1) * P, :], ob[:]
                        )

                pending = []
                for hp in range(4):      # head pair (2hp, 2hp+1)
                    for i in range(NCH):  # query chunk
                        ps = []
                        for sub in range(2):  # head within pair
                            p0 = sub * 64
                            psc = sc_psum.tile([P, S], F32, tag="sc")
                            for nn in range(2):
                                nc.tensor.matmul(
                                    psc[:, nn * 512 : (nn + 1) * 512],
                                    qT_sb[p0 : p0 + 64, hp, i * P : (i + 1) * P],
                                    kT_sb[p0 : p0 + 64, hp, nn * 512 : (nn + 1) * 512],
                                    start=True, stop=True,
                                )
                            ps.append(psc)
                        # rowwise min/max -> alpha/beta for fused sigmoid
                        # layout: [maxA, maxB, minA, minB, betaA, betaB, alpA, alpB]
                        st = sstat.tile([P, 8], F32, tag="st")
                        for sub in range(2):
                            nc.vector.tensor_reduce(
                                st[:, sub : sub + 1], ps[sub][:],
                                axis=mybir.AxisListType.X, op=mybir.AluOpType.max,
                            )
                            nc.vector.tensor_reduce(
                                st[:, 2 + sub : 3 + sub], ps[sub][:],
                                axis=mybir.AxisListType.X, op=mybir.AluOpType.min,
                            )
                        mx = st[:, 0:2]
                        mn = st[:, 2:4]
                        rng = st[:, 4:6]
                        alp = st[:, 6:8]
                        nc.vector.tensor_sub(rng, mx, mn)
                        nc.vector.reciprocal(rng, rng)
                        nc.vector.tensor_scalar_mul(alp, rng, 10.0)
                        # beta = -(mn*alpha) - 5  (reuse rng slot)
                        nc.vector.tensor_mul(rng, mn, alp)
                        nc.vector.tensor_scalar(
                            out=rng, in0=rng, scalar1=-1.0, scalar2=-5.0,
                            op0=mybir.AluOpType.mult, op1=mybir.AluOpType.add,
                        )
                        wts = []
                        for sub in range(2):
                            w_t = w_pool.tile([P, S], BF, tag="w")
                            nc.scalar.activation(
                                out=w_t[:], in_=ps[sub][:],
                                func=mybir.ActivationFunctionType.Sigmoid,
                                bias=st[:, 4 + sub : 5 + sub],
                                scale=st[:, 6 + sub : 7 + sub],
                            )
                            wt_t = wt_pool.tile([P, NCH, P], BF, tag="wt")
                            nc.sync.dma_start_transpose(wt_t[:], w_t[:])
                            if debug and hp == 0 and i == 0 and sub == 0:
                                nc.sync.dma_start(dbg_w_d[:], wt_t[:])
                                nc.sync.dma_start(dbg_wp_d[:], w_t[:])
                                dbg_sc_t = w_pool.tile([P, S], F32, tag="dbgsc")
                                nc.vector.tensor_copy(dbg_sc_t[:], ps[sub][:])
                                nc.sync.dma_start(dbg_sc_d[:], dbg_sc_t[:])
                                nc.sync.dma_start(dbg_st_d[:], st[:])
                            wts.append(wt_t)
                        pending.append((hp, i, wts))
                        if len(pending) > 1:
                            out_stage(pending.pop(0))
                for pp_ in pending:
                    out_stage(pp_)

    nc.compile()
    _BUILD_CACHE[(use_beta,)] = nc
    return nc


def _prep_inputs(x, emb_table, gamma, beta, Wq, Wk, Wv, use_beta):
    x = np.asarray(x)
    gamma = np.asarray(gamma, dtype=np.float32)
    beta = np.asarray(beta, dtype=np.float32)
    Wq = np.asarray(Wq, dtype=np.float32)
    Wk = np.asarray(Wk, dtype=np.float32)
    Wv = np.asarray(Wv, dtype=np.float32)
    emb = np.ascontiguousarray(np.asarray(emb_table, dtype=np.float32))

    # W'[h,a,e] = W[h,a,e] * gamma[e]; layouts [e, h*ad+a]
    wqt = np.ascontiguousarray(
        (Wq * gamma[None, None, :]).reshape(H * A, E).T.astype(BF16)
    )
    wkt = np.ascontiguousarray(
        (Wk * gamma[None, None, :]).reshape(H * A, E).T.astype(BF16)
    )
    wvt = np.ascontiguousarray(
        (Wv * gamma[None, None, :]).reshape(H * E, E).T.astype(BF16)
    )
    idn = np.eye(P, dtype=np.float32).astype(BF16)

    consts = dict(emb=emb, wqt=wqt, wkt=wkt, wvt=wvt, idn=idn)
    if use_beta:
        qb = (Wq.reshape(H * A, E) @ beta).astype(np.float32)   # [512]
        kb = (Wk.reshape(H * A, E) @ beta).astype(np.float32)
        vb = (Wv.reshape(H * E, E) @ beta).astype(np.float32)   # [4096]
        consts["qb"] = np.ascontiguousarray(qb.reshape(4, P).T)
        consts["kb"] = np.ascontiguousarray(kb.reshape(4, P).T)
        consts["vb"] = vb.reshape(1, H * E)

    in_maps = []
    for b in range(B):
        xi = x[b].astype(np.int64)
        idx16 = np.ascontiguousarray(
            xi.reshape(S // 16, 16).T.astype(np.int16)
        )  # [16, 64]; token j of chunk c sits at [j%16, 8c + j//16]
        idx_full = np.ascontiguousarray(np.tile(idx16, (8, 1)))  # [128, 64]
        in_maps.append(dict(idx=idx_full, **consts))
    return in_maps


def kernel(x, emb_table, gamma, beta, Wq, Wk, Wv):
    global LAST_RESULTS
    beta_arr = np.asarray(beta, dtype=np.float32)
    use_beta = bool(np.any(beta_arr != 0.0))

    nc = build_nc(use_beta)
    in_maps = _prep_inputs(x, emb_table, gamma, beta, Wq, Wk, Wv, use_beta)

    trace = os.environ.get("KERNEL_TRACE", "0") == "1"
    res = run_bass_kernel_spmd(
        nc, in_maps, core_ids=list(range(B)), trace=trace
    )
    LAST_RESULTS = res

    out = np.stack([np.asarray(res.results[b]["out"]) for b in range(B)], axis=0)
    return out.astype(np.float32)


if __name__ == "__main__":
    rng = np.random.default_rng(0)
    x = rng.integers(0, VOCAB, size=(B, S), dtype=np.int32)
    emb = rng.standard_normal((VOCAB, E), dtype=np.float32)
    gamma = np.ones(E, np.float32)
    beta = np.zeros(E, np.float32)
    Wq = rng.random((H, A, E), dtype=np.float32)
    Wk = rng.random((H, A, E), dtype=np.float32)
    Wv = rng.random((H, E, E), dtype=np.float32)
    out = kernel(x, emb, gamma, beta, Wq, Wk, Wv)
    print(out.shape, out.dtype)


# revision 18
# speedup vs baseline: 1.0271x; 1.0271x over previous
"""Trainium2 Bass kernel for EruSelfAttentionModel.

Math (reference, simplified):
  e  = emb_table[x]                                  # [B,S,E] gather
  h  = LayerNorm(e) * gamma + beta                   # over E
  q  = einsum('hae,bse->bhsa', Wq, h); k likewise    # A=64 per head
  v  = einsum('hve,bse->bhsv', Wv, h)                # v-dim = E
  scores = q @ k^T / sqrt(E)
  sn = (scores - min) / (max - min)  (rowwise)
  softmax_sel = 1 - max(sn) == 0 exactly  =>  weights = sigmoid(10*sn - 5)
  out = weights @ v                                  # [B,H,S,E]

Key identities used:
  - sn is invariant to positive rescaling of scores => the 1/sqrt(E) scale
    can be dropped entirely.
  - weights = sigmoid(alpha * scores + beta_row) with per-row
    alpha = 10/(mx-mn), beta_row = -10*mn/(mx-mn) - 5  => single fused
    ScalarE activation pass (per-partition scale/bias APs).

Sharding: data-parallel over batch; core b computes batch b fully.
"""

import os
import sys

sys.path.insert(0, "/opt/trn_rl_repo")

import numpy as np
import ml_dtypes

import concourse.bass as bass
import concourse.bacc as bacc
import concourse.tile as tile
from concourse import mybir
from concourse.bass_utils import run_bass_kernel_spmd
import concourse.bass_utils as _bass_utils

# Let walrus overlap LDWEIGHTS with in-flight matmuls (the PE pulls LDW for
# the background weight buffer ahead); without it every matmul serializes
# behind its weight load (~+160ns per matmul).
if not getattr(_bass_utils, "_ldw_patched", False):
    _orig_run_command = _bass_utils.run_command

    def _patched_run_command(argv, **kwargs):
        if os.environ.get("KERNEL_LDW_OPT", "1") == "1":
            argv = [
                a.replace("--enable-ldw-opt=false", "--enable-ldw-opt=true")
                if isinstance(a, str) else a
                for a in argv
            ]
        return _orig_run_command(argv, **kwargs)

    _bass_utils.run_command = _patched_run_command
    _bass_utils._ldw_patched = True

BF16 = ml_dtypes.bfloat16

VOCAB, E, A, H = 32000, 512, 64, 8
B, S = 8, 1024
P = 128                 # partitions
NCH = S // P            # 8 token chunks
EC = E // P             # 4 embedding chunks
LN_EPS = 1e-5

F32 = mybir.dt.float32
BF = mybir.dt.bfloat16
I16 = mybir.dt.int16

_BUILD_CACHE = {}
LAST_RESULTS = None     # test.py reads exec_time_ns from here


def build_nc(use_beta: bool, debug: bool = False):
    if (use_beta, debug) in _BUILD_CACHE:
        return _BUILD_CACHE[(use_beta, debug)]

    nc = bacc.Bacc("TRN2", target_bir_lowering=False, num_devices=8)

    idx_d = nc.declare_dram_parameter("idx", [P, S // 16], I16, isOutput=False)
    emb_d = nc.declare_dram_parameter("emb", [VOCAB, E], F32, isOutput=False)
    wqt_d = nc.declare_dram_parameter("wqt", [E, H * A], BF, isOutput=False)
    wkt_d = nc.declare_dram_parameter("wkt", [E, H * A], BF, isOutput=False)
    wvt_d = nc.declare_dram_parameter("wvt", [E, H * E], BF, isOutput=False)
    idn_d = nc.declare_dram_parameter("idn", [P, P], BF, isOutput=False)
    if use_beta:
        qb_d = nc.declare_dram_parameter("qb", [P, 4], F32, isOutput=False)
        kb_d = nc.declare_dram_parameter("kb", [P, 4], F32, isOutput=False)
        vb_d = nc.declare_dram_parameter("vb", [1, H * E], F32, isOutput=False)
    out_d = nc.declare_dram_parameter("out", [H, S, E], BF, isOutput=True)
    if debug:
        dbg_ht_d = nc.declare_dram_parameter("dbg_ht", [P, EC, S], BF, isOutput=True)
        dbg_qt_d = nc.declare_dram_parameter("dbg_qt", [P, EC, S], BF, isOutput=True)
        dbg_kt_d = nc.declare_dram_parameter("dbg_kt", [P, EC, S], BF, isOutput=True)
        dbg_vh_d = nc.declare_dram_parameter("dbg_vh", [P, NCH, H * E], BF, isOutput=True)
        dbg_w_d = nc.declare_dram_parameter("dbg_w", [P, NCH, P], BF, isOutput=True)
        dbg_wp_d = nc.declare_dram_parameter("dbg_wp", [P, S], BF, isOutput=True)
        dbg_sc_d = nc.declare_dram_parameter("dbg_sc", [P, S], F32, isOutput=True)
        dbg_st_d = nc.declare_dram_parameter("dbg_st", [P, 8], F32, isOutput=True)

    with tile.TileContext(nc) as tc:
        with tc.tile_pool(name="consts", bufs=1) as consts:
            idx_sb = consts.tile([P, S // 16], I16)
            nc.sync.dma_start(idx_sb[:], idx_d[:])
            idn_sb = consts.tile([P, P], BF)
            nc.sync.dma_start(idn_sb[:], idn_d[:])
            wqt_sb = consts.tile([P, EC, H * A], BF)
            nc.sync.dma_start(
                wqt_sb[:], wqt_d.ap().rearrange("(ec p) j -> p ec j", p=P)
            )
            wkt_sb = consts.tile([P, EC, H * A], BF)
            nc.sync.dma_start(
                wkt_sb[:], wkt_d.ap().rearrange("(ec p) j -> p ec j", p=P)
            )
            wvt_sb = consts.tile([P, EC, H * E], BF)
            nc.sync.dma_start(
                wvt_sb[:], wvt_d.ap().rearrange("(ec p) j -> p ec j", p=P)
            )
            eps_sb = consts.tile([P, 1], F32)
            nc.vector.memset(eps_sb[:], LN_EPS)

            if use_beta:
                qb_sb = consts.tile([P, 4], F32)
                nc.sync.dma_start(qb_sb[:], qb_d[:])
                kb_sb = consts.tile([P, 4], F32)
                nc.sync.dma_start(kb_sb[:], kb_d[:])
                vb_sb = consts.tile([P, H * E], F32)
                vb_bcast = bass.AP(
                    tensor=vb_d, offset=0, ap=[[0, P], [1, H * E]]
                )
                nc.sync.dma_start(vb_sb[:], vb_bcast)

            # persistent activations
            hT_sb = consts.tile([P, EC, S], BF)       # hT[e%128, e//128, s]
            qT_sb = consts.tile([P, EC, S], BF)       # qT[ha%128, ha//128, s]
            kT_sb = consts.tile([P, EC, S], BF)
            vh_sb = consts.tile([P, NCH, H * E], BF)  # vh[p, c, v] = V[8p+c, v]

            # ---------------- phase A: gather + LN + transpose ----------------
            with (
                tc.tile_pool(name="e_pool", bufs=1) as e_pool,
                tc.tile_pool(name="h_pool", bufs=3) as h_pool,
                tc.tile_pool(name="st_pool", bufs=8) as st_pool,
                tc.tile_pool(name="pt_psum", bufs=4, space="PSUM") as pt_psum,
            ):
                e_ts = []
                mv = st_pool.tile([P, NCH, 2], F32, tag="mv")
                for c in range(NCH):
                    e_t = e_pool.tile([P, 1, E], F32, tag=f"e{c}")
                    nc.gpsimd.dma_gather(
                        e_t[:], emb_d.ap(), idx_sb[:, 8 * c : 8 * (c + 1)],
                        P, P, E,
                    )
                    e_ts.append(e_t)
                    stt = st_pool.tile([P, 6], F32, tag="bn")
                    nc.vector.bn_stats(stt[:], e_t[:, 0, :])
                    nc.vector.bn_aggr(mv[:, c, :], stt[:])
                # batched inv-std: var <- 1/sqrt(var+eps) for all 8 chunks
                var_ap = mv[:, :, 1]
                nc.scalar.activation(
                    out=var_ap, in_=var_ap,
                    func=mybir.ActivationFunctionType.Sqrt,
                    bias=eps_sb[:, 0:1], scale=1.0,
                )
                nc.vector.reciprocal(var_ap, var_ap)
                for c in range(NCH):
                    h_t = h_pool.tile([P, E], BF)
                    nc.vector.tensor_scalar(
                        out=h_t[:], in0=e_ts[c][:, 0, :],
                        scalar1=mv[:, c, 0:1], scalar2=mv[:, c, 1:2],
                        op0=mybir.AluOpType.subtract, op1=mybir.AluOpType.mult,
                    )
                    for ec in range(EC):
                        pt = pt_psum.tile([P, P], BF)
                        nc.tensor.transpose(
                            pt[:], h_t[:, ec * P : (ec + 1) * P], idn_sb[:]
                        )
                        nc.any.tensor_copy(
                            hT_sb[:, ec, c * P : (c + 1) * P], pt[:]
                        )

            # ---------------- phase B: projections ----------------
            with tc.tile_pool(name="proj_psum", bufs=4, space="PSUM") as proj_psum:
                for c in range(NCH):  # t-groups: partition p holds t = c*128+p
                    for vp in range(4):  # 1024-wide v pairs
                        pv = proj_psum.tile([P, S], F32, tag="pp")
                        for ec in range(EC):
                            lhsT = hT_sb[:, ec, c * P : (c + 1) * P]
                            for nn in range(2):
                                lo = vp * 1024 + nn * 512
                                nc.tensor.matmul(
                                    pv[:, nn * 512 : (nn + 1) * 512],
                                    lhsT,
                                    wvt_sb[:, ec, lo : lo + 512],
                                    start=(ec == 0), stop=(ec == EC - 1),
                                )
                        if use_beta:
                            nc.vector.tensor_add(
                                out=pv[:], in0=pv[:],
                                in1=vb_sb[:, vp * 1024 : (vp + 1) * 1024],
                            )
                        nc.any.tensor_copy(
                            vh_sb[:, c, vp * 1024 : (vp + 1) * 1024], pv[:]
                        )

                for name, w_sb, t_sb, b_sb in (
                    ("q", wqt_sb, qT_sb, "qb"),
                    ("k", wkt_sb, kT_sb, "kb"),
                ):
                    for sl in range(4):  # 128-wide (h,a) slices = head pairs
                        pq = proj_psum.tile([P, S], F32, tag="pp")
                        for ec in range(EC):
                            lhsT = w_sb[:, ec, sl * P : (sl + 1) * P]
                            for nn in range(2):
                                nc.tensor.matmul(
                                    pq[:, nn * 512 : (nn + 1) * 512],
                                    lhsT,
                                    hT_sb[:, ec, nn * 512 : (nn + 1) * 512],
                                    start=(ec == 0), stop=(ec == EC - 1),
                                )
                        if use_beta:
                            bb = qb_sb if name == "q" else kb_sb
                            nc.vector.tensor_scalar_add(
                                out=pq[:], in0=pq[:], scalar1=bb[:, sl : sl + 1]
                            )
                        nc.any.tensor_copy(t_sb[:, sl, :], pq[:])

            if debug:
                nc.sync.dma_start(dbg_ht_d[:], hT_sb[:])
                nc.sync.dma_start(dbg_qt_d[:], qT_sb[:])
                nc.sync.dma_start(dbg_kt_d[:], kT_sb[:])
                nc.sync.dma_start(dbg_vh_d[:], vh_sb[:])

            # ---------------- phase C: attention ----------------
            with (
                tc.tile_pool(name="sc_psum", bufs=3, space="PSUM") as sc_psum,
                tc.tile_pool(name="out_psum", bufs=2, space="PSUM") as out_psum,
                tc.tile_pool(name="sstat", bufs=8) as sstat,
                tc.tile_pool(name="w_pool", bufs=6) as w_pool,
                tc.tile_pool(name="wraw_pool", bufs=6) as wraw_pool,
                tc.tile_pool(name="wt_pool", bufs=6) as wt_pool,
                tc.tile_pool(name="ob_pool", bufs=4) as ob_pool,
            ):
                def out_stage(args):
                    hp_, i_, wts = args
                    for sub in range(2):
                        h_idx = 2 * hp_ + sub
                        wt_t = wts[sub]
                        po = out_psum.tile([P, E], F32, tag="po")
                        for cc in range(NCH):
                            nc.tensor.matmul(
                                po[:],
                                wt_t[:, cc, :],
                                vh_sb[:, cc, h_idx * E : (h_idx + 1) * E],
                                start=(cc == 0), stop=(cc == NCH - 1),
                            )
                        ob = ob_pool.tile([P, E], BF, tag="ob")
                        nc.scalar.copy(ob[:], po[:])
                        nc.scalar.dma_start(
                            out_d[h_idx, i_ * P : (i_ + 1) * P, :], ob[:]
                        )

                pending = []
                for hp in range(4):      # head pair (2hp, 2hp+1)
                    for i in range(NCH):  # query chunk
                        ps = []
                        for sub in range(2):  # head within pair
                            p0 = sub * 64
                            psc = sc_psum.tile([P, S], F32, tag="sc")
                            for nn in range(2):
                                nc.tensor.matmul(
                                    psc[:, nn * 512 : (nn + 1) * 512],
                                    qT_sb[p0 : p0 + 64, hp, i * P : (i + 1) * P],
                                    kT_sb[p0 : p0 + 64, hp, nn * 512 : (nn + 1) * 512],
                                    start=True, stop=True,
                                )
                            wraw = wraw_pool.tile([P, S], F32, tag="wr")
                            nc.scalar.copy(wraw[:], psc[:])
                            ps.append(wraw)
                        # rowwise min/max -> alpha/beta for fused sigmoid
                        # layout: [maxA, maxB, minA, minB, betaA, betaB, alpA, alpB]
                        st = sstat.tile([P, 8], F32, tag="st")
                        for sub in range(2):
                            nc.vector.tensor_reduce(
                                st[:, sub : sub + 1], ps[sub][:],
                                axis=mybir.AxisListType.X, op=mybir.AluOpType.max,
                            )
                            nc.vector.tensor_reduce(
                                st[:, 2 + sub : 3 + sub], ps[sub][:],
                                axis=mybir.AxisListType.X, op=mybir.AluOpType.min,
                            )
                        mx = st[:, 0:2]
                        mn = st[:, 2:4]
                        rng = st[:, 4:6]
                        alp = st[:, 6:8]
                        nc.vector.tensor_sub(rng, mx, mn)
                        nc.vector.reciprocal(rng, rng)
                        nc.vector.tensor_scalar_mul(alp, rng, 10.0)
                        # beta = -(mn*alpha) - 5  (reuse rng slot)
                        nc.vector.tensor_mul(rng, mn, alp)
                        nc.vector.tensor_scalar(
                            out=rng, in0=rng, scalar1=-1.0, scalar2=-5.0,
                            op0=mybir.AluOpType.mult, op1=mybir.AluOpType.add,
                        )
                        wts = []
                        for sub in range(2):
                            w_t = w_pool.tile([P, S], BF, tag="w")
                            nc.scalar.activation(
                                out=w_t[:], in_=ps[sub][:],
                                func=mybir.ActivationFunctionType.Sigmoid,
                                bias=st[:, 4 + sub : 5 + sub],
                                scale=st[:, 6 + sub : 7 + sub],
                            )
                            wt_t = wt_pool.tile([P, NCH, P], BF, tag="wt")
                            nc.sync.dma_start_transpose(wt_t[:], w_t[:])
                            if debug and hp == 0 and i == 0 and sub == 0:
                                nc.sync.dma_start(dbg_w_d[:], wt_t[:])
                                nc.sync.dma_start(dbg_wp_d[:], w_t[:])
                                dbg_sc_t = w_pool.tile([P, S], F32, tag="dbgsc")
                                nc.vector.tensor_copy(dbg_sc_t[:], ps[sub][:])
                                nc.sync.dma_start(dbg_sc_d[:], dbg_sc_t[:])
                                nc.sync.dma_start(dbg_st_d[:], st[:])
                            wts.append(wt_t)
                        pending.append((hp, i, wts))
                        if len(pending) > 1:
                            out_stage(pending.pop(0))
                for pp_ in pending:
                    out_stage(pp_)

    nc.compile()
    _BUILD_CACHE[(use_beta,)] = nc
    return nc


def _prep_inputs(x, emb_table, gamma, beta, Wq, Wk, Wv, use_beta):
    x = np.asarray(x)
    gamma = np.asarray(gamma, dtype=np.float32)
    beta = np.asarray(beta, dtype=np.float32)
    Wq = np.asarray(Wq, dtype=np.float32)
    Wk = np.asarray(Wk, dtype=np.float32)
    Wv = np.asarray(Wv, dtype=np.float32)
    emb = np.ascontiguousarray(np.asarray(emb_table, dtype=np.float32))

    # W'[h,a,e] = W[h,a,e] * gamma[e]; layouts [e, h*ad+a]
    wqt = np.ascontiguousarray(
        (Wq * gamma[None, None, :]).reshape(H * A, E).T.astype(BF16)
    )
    wkt = np.ascontiguousarray(
        (Wk * gamma[None, None, :]).reshape(H * A, E).T.astype(BF16)
    )
    wvt = np.ascontiguousarray(
        (Wv * gamma[None, None, :]).reshape(H * E, E).T.astype(BF16)
    )
    idn = np.eye(P, dtype=np.float32).astype(BF16)

    consts = dict(emb=emb, wqt=wqt, wkt=wkt, wvt=wvt, idn=idn)
    if use_beta:
        qb = (Wq.reshape(H * A, E) @ beta).astype(np.float32)   # [512]
        kb = (Wk.reshape(H * A, E) @ beta).astype(np.float32)
        vb = (Wv.reshape(H * E, E) @ beta).astype(np.float32)   # [4096]
        consts["qb"] = np.ascontiguousarray(qb.reshape(4, P).T)
        consts["kb"] = np.ascontiguousarray(kb.reshape(4, P).T)
        consts["vb"] = vb.reshape(1, H * E)

    in_maps = []
    for b in range(B):
        xi = x[b].astype(np.int64)
        idx16 = np.ascontiguousarray(
            xi.reshape(S // 16, 16).T.astype(np.int16)
        )  # [16, 64]; token j of chunk c sits at [j%16, 8c + j//16]
        idx_full = np.ascontiguousarray(np.tile(idx16, (8, 1)))  # [128, 64]
        in_maps.append(dict(idx=idx_full, **consts))
    return in_maps


def kernel(x, emb_table, gamma, beta, Wq, Wk, Wv):
    global LAST_RESULTS
    beta_arr = np.asarray(beta, dtype=np.float32)
    use_beta = bool(np.any(beta_arr != 0.0))

    nc = build_nc(use_beta)
    in_maps = _prep_inputs(x, emb_table, gamma, beta, Wq, Wk, Wv, use_beta)

    trace = os.environ.get("KERNEL_TRACE", "0") == "1"
    res = run_bass_kernel_spmd(
        nc, in_maps, core_ids=list(range(B)), trace=trace
    )
    LAST_RESULTS = res

    out = np.stack([np.asarray(res.results[b]["out"]) for b in range(B)], axis=0)
    return out.astype(np.float32)


if __name__ == "__main__":
    rng = np.random.default_rng(0)
    x = rng.integers(0, VOCAB, size=(B, S), dtype=np.int32)
    emb = rng.standard_normal((VOCAB, E), dtype=np.float32)
    gamma = np.ones(E, np.float32)
    beta = np.zeros(E, np.float32)
    Wq = rng.random((H, A, E), dtype=np.float32)
    Wk = rng.random((H, A, E), dtype=np.float32)
    Wv = rng.random((H, E, E), dtype=np.float32)
    out = kernel(x, emb, gamma, beta, Wq, Wk, Wv)
    print(out.shape, out.dtype)


# revision 19
# speedup vs baseline: 1.1192x; 1.0896x over previous
"""Trainium2 Bass kernel for EruSelfAttentionModel.

Math (reference, simplified):
  e  = emb_table[x]                                  # [B,S,E] gather
  h  = LayerNorm(e) * gamma + beta                   # over E
  q  = einsum('hae,bse->bhsa', Wq, h); k likewise    # A=64 per head
  v  = einsum('hve,bse->bhsv', Wv, h)                # v-dim = E
  scores = q @ k^T / sqrt(E)
  sn = (scores - min) / (max - min)  (rowwise)
  softmax_sel = 1 - max(sn) == 0 exactly  =>  weights = sigmoid(10*sn - 5)
  out = weights @ v                                  # [B,H,S,E]

Key identities used:
  - sn is invariant to positive rescaling of scores => the 1/sqrt(E) scale
    can be dropped entirely.
  - weights = sigmoid(alpha * scores + beta_row) with per-row
    alpha = 10/(mx-mn), beta_row = -10*mn/(mx-mn) - 5  => single fused
    ScalarE activation pass (per-partition scale/bias APs).

Sharding: data-parallel over batch; core b computes batch b fully.
"""

import os
import sys

sys.path.insert(0, "/opt/trn_rl_repo")

import numpy as np
import ml_dtypes

import concourse.bass as bass
import concourse.bacc as bacc
import concourse.tile as tile
from concourse import mybir
from concourse.bass_utils import run_bass_kernel_spmd
import concourse.bass_utils as _bass_utils

# Let walrus overlap LDWEIGHTS with in-flight matmuls (the PE pulls LDW for
# the background weight buffer ahead); without it every matmul serializes
# behind its weight load (~+160ns per matmul).
if not getattr(_bass_utils, "_ldw_patched", False):
    _orig_run_command = _bass_utils.run_command

    def _patched_run_command(argv, **kwargs):
        if os.environ.get("KERNEL_LDW_OPT", "1") == "1":
            argv = [
                a.replace("--enable-ldw-opt=false", "--enable-ldw-opt=true")
                if isinstance(a, str) else a
                for a in argv
            ]
        return _orig_run_command(argv, **kwargs)

    _bass_utils.run_command = _patched_run_command
    _bass_utils._ldw_patched = True

BF16 = ml_dtypes.bfloat16

VOCAB, E, A, H = 32000, 512, 64, 8
B, S = 8, 1024
P = 128                 # partitions
NCH = S // P            # 8 token chunks
EC = E // P             # 4 embedding chunks
LN_EPS = 1e-5

F32 = mybir.dt.float32
BF = mybir.dt.bfloat16
I16 = mybir.dt.int16

_BUILD_CACHE = {}
LAST_RESULTS = None     # test.py reads exec_time_ns from here


def build_nc(use_beta: bool, debug: bool = False):
    if (use_beta, debug) in _BUILD_CACHE:
        return _BUILD_CACHE[(use_beta, debug)]

    nc = bacc.Bacc("TRN2", target_bir_lowering=False, num_devices=8)

    idx_d = nc.declare_dram_parameter("idx", [P, S // 16], I16, isOutput=False)
    emb_d = nc.declare_dram_parameter("emb", [VOCAB, E], F32, isOutput=False)
    wqt_d = nc.declare_dram_parameter("wqt", [E, H * A], BF, isOutput=False)
    wkt_d = nc.declare_dram_parameter("wkt", [E, H * A], BF, isOutput=False)
    wvt_d = nc.declare_dram_parameter("wvt", [E, H * E], BF, isOutput=False)
    idn_d = nc.declare_dram_parameter("idn", [P, P], BF, isOutput=False)
    if use_beta:
        qb_d = nc.declare_dram_parameter("qb", [P, 4], F32, isOutput=False)
        kb_d = nc.declare_dram_parameter("kb", [P, 4], F32, isOutput=False)
        vb_d = nc.declare_dram_parameter("vb", [1, H * E], F32, isOutput=False)
    out_d = nc.declare_dram_parameter("out", [H, S, E], BF, isOutput=True)
    if debug:
        dbg_ht_d = nc.declare_dram_parameter("dbg_ht", [P, EC, S], BF, isOutput=True)
        dbg_qt_d = nc.declare_dram_parameter("dbg_qt", [P, EC, S], BF, isOutput=True)
        dbg_kt_d = nc.declare_dram_parameter("dbg_kt", [P, EC, S], BF, isOutput=True)
        dbg_vh_d = nc.declare_dram_parameter("dbg_vh", [P, NCH, H * E], BF, isOutput=True)
        dbg_w_d = nc.declare_dram_parameter("dbg_w", [P, NCH, P], BF, isOutput=True)
        dbg_wp_d = nc.declare_dram_parameter("dbg_wp", [P, S], BF, isOutput=True)
        dbg_sc_d = nc.declare_dram_parameter("dbg_sc", [P, S], F32, isOutput=True)
        dbg_st_d = nc.declare_dram_parameter("dbg_st", [P, 8], F32, isOutput=True)

    with tile.TileContext(nc) as tc:
        with tc.tile_pool(name="consts", bufs=1) as consts:
            idx_sb = consts.tile([P, S // 16], I16)
            nc.sync.dma_start(idx_sb[:], idx_d[:])
            idn_sb = consts.tile([P, P], BF)
            nc.sync.dma_start(idn_sb[:], idn_d[:])
            wqt_sb = consts.tile([P, EC, H * A], BF)
            nc.sync.dma_start(
                wqt_sb[:], wqt_d.ap().rearrange("(ec p) j -> p ec j", p=P)
            )
            wkt_sb = consts.tile([P, EC, H * A], BF)
            nc.sync.dma_start(
                wkt_sb[:], wkt_d.ap().rearrange("(ec p) j -> p ec j", p=P)
            )
            wvt_sb = consts.tile([P, EC, H * E], BF)
            nc.sync.dma_start(
                wvt_sb[:], wvt_d.ap().rearrange("(ec p) j -> p ec j", p=P)
            )
            eps_sb = consts.tile([P, 1], F32)
            nc.vector.memset(eps_sb[:], LN_EPS)

            if use_beta:
                qb_sb = consts.tile([P, 4], F32)
                nc.sync.dma_start(qb_sb[:], qb_d[:])
                kb_sb = consts.tile([P, 4], F32)
                nc.sync.dma_start(kb_sb[:], kb_d[:])
                vb_sb = consts.tile([P, H * E], F32)
                vb_bcast = bass.AP(
                    tensor=vb_d, offset=0, ap=[[0, P], [1, H * E]]
                )
                nc.sync.dma_start(vb_sb[:], vb_bcast)

            # persistent activations
            hT_sb = consts.tile([P, EC, S], BF)       # hT[e%128, e//128, s]
            qT_sb = consts.tile([P, EC, S], BF)       # qT[ha%128, ha//128, s]
            kT_sb = consts.tile([P, EC, S], BF)
            vh_sb = consts.tile([P, NCH, H * E], BF)  # vh[p, c, v] = V[8p+c, v]

            # ---------------- phase A: gather + LN + transpose ----------------
            with (
                tc.tile_pool(name="e_pool", bufs=1) as e_pool,
                tc.tile_pool(name="h_pool", bufs=3) as h_pool,
                tc.tile_pool(name="st_pool", bufs=8) as st_pool,
                tc.tile_pool(name="pt_psum", bufs=4, space="PSUM") as pt_psum,
            ):
                e_ts = []
                mv = st_pool.tile([P, NCH, 2], F32, tag="mv")
                for c in range(NCH):
                    e_t = e_pool.tile([P, 1, E], F32, tag=f"e{c}")
                    nc.gpsimd.dma_gather(
                        e_t[:], emb_d.ap(), idx_sb[:, 8 * c : 8 * (c + 1)],
                        P, P, E,
                    )
                    e_ts.append(e_t)
                    stt = st_pool.tile([P, 6], F32, tag="bn")
                    nc.vector.bn_stats(stt[:], e_t[:, 0, :])
                    nc.vector.bn_aggr(mv[:, c, :], stt[:])
                # batched inv-std: var <- 1/sqrt(var+eps) for all 8 chunks
                var_ap = mv[:, :, 1]
                nc.scalar.activation(
                    out=var_ap, in_=var_ap,
                    func=mybir.ActivationFunctionType.Sqrt,
                    bias=eps_sb[:, 0:1], scale=1.0,
                )
                nc.vector.reciprocal(var_ap, var_ap)
                for c in range(NCH):
                    h_t = h_pool.tile([P, E], BF)
                    nc.vector.tensor_scalar(
                        out=h_t[:], in0=e_ts[c][:, 0, :],
                        scalar1=mv[:, c, 0:1], scalar2=mv[:, c, 1:2],
                        op0=mybir.AluOpType.subtract, op1=mybir.AluOpType.mult,
                    )
                    for ec in range(EC):
                        pt = pt_psum.tile([P, P], BF)
                        nc.tensor.transpose(
                            pt[:], h_t[:, ec * P : (ec + 1) * P], idn_sb[:]
                        )
                        nc.any.tensor_copy(
                            hT_sb[:, ec, c * P : (c + 1) * P], pt[:]
                        )

            # ---------------- phase B: projections ----------------
            with tc.tile_pool(name="proj_psum", bufs=4, space="PSUM") as proj_psum:
                for c in range(NCH):  # t-groups: partition p holds t = c*128+p
                    for vp in range(4):  # 1024-wide v pairs
                        pv = proj_psum.tile([P, S], F32, tag="pp")
                        for ec in range(EC):
                            lhsT = hT_sb[:, ec, c * P : (c + 1) * P]
                            for nn in range(2):
                                lo = vp * 1024 + nn * 512
                                nc.tensor.matmul(
                                    pv[:, nn * 512 : (nn + 1) * 512],
                                    lhsT,
                                    wvt_sb[:, ec, lo : lo + 512],
                                    start=(ec == 0), stop=(ec == EC - 1),
                                )
                        if use_beta:
                            nc.vector.tensor_add(
                                out=pv[:], in0=pv[:],
                                in1=vb_sb[:, vp * 1024 : (vp + 1) * 1024],
                            )
                        nc.any.tensor_copy(
                            vh_sb[:, c, vp * 1024 : (vp + 1) * 1024], pv[:]
                        )

                for name, w_sb, t_sb, b_sb in (
                    ("q", wqt_sb, qT_sb, "qb"),
                    ("k", wkt_sb, kT_sb, "kb"),
                ):
                    for sl in range(4):  # 128-wide (h,a) slices = head pairs
                        pq = proj_psum.tile([P, S], F32, tag="pp")
                        for ec in range(EC):
                            lhsT = w_sb[:, ec, sl * P : (sl + 1) * P]
                            for nn in range(2):
                                nc.tensor.matmul(
                                    pq[:, nn * 512 : (nn + 1) * 512],
                                    lhsT,
                                    hT_sb[:, ec, nn * 512 : (nn + 1) * 512],
                                    start=(ec == 0), stop=(ec == EC - 1),
                                )
                        if use_beta:
                            bb = qb_sb if name == "q" else kb_sb
                            nc.vector.tensor_scalar_add(
                                out=pq[:], in0=pq[:], scalar1=bb[:, sl : sl + 1]
                            )
                        nc.any.tensor_copy(t_sb[:, sl, :], pq[:])

            if debug:
                nc.sync.dma_start(dbg_ht_d[:], hT_sb[:])
                nc.sync.dma_start(dbg_qt_d[:], qT_sb[:])
                nc.sync.dma_start(dbg_kt_d[:], kT_sb[:])
                nc.sync.dma_start(dbg_vh_d[:], vh_sb[:])

            # ---------------- phase C: attention ----------------
            with (
                tc.tile_pool(name="sc_psum", bufs=3, space="PSUM") as sc_psum,
                tc.tile_pool(name="out_psum", bufs=2, space="PSUM") as out_psum,
                tc.tile_pool(name="sstat", bufs=8) as sstat,
                tc.tile_pool(name="w_pool", bufs=6) as w_pool,
                tc.tile_pool(name="wraw_pool", bufs=6) as wraw_pool,
                tc.tile_pool(name="wt_pool", bufs=6) as wt_pool,
                tc.tile_pool(name="ob_pool", bufs=4) as ob_pool,
            ):
                def out_stage(args):
                    hp_, i_, wts = args
                    for sub in range(2):
                        h_idx = 2 * hp_ + sub
                        wt_t = wts[sub]
                        po = out_psum.tile([P, E], F32, tag="po")
                        for cc in range(NCH):
                            nc.tensor.matmul(
                                po[:],
                                wt_t[:, cc, :],
                                vh_sb[:, cc, h_idx * E : (h_idx + 1) * E],
                                start=(cc == 0), stop=(cc == NCH - 1),
                            )
                        ob = ob_pool.tile([P, E], BF, tag="ob")
                        nc.scalar.copy(ob[:], po[:])
                        nc.sync.dma_start(
                            out_d[h_idx, i_ * P : (i_ + 1) * P, :], ob[:]
                        )

                pending = []
                for hp in range(4):      # head pair (2hp, 2hp+1)
                    for i in range(NCH):  # query chunk
                        ps = []
                        for sub in range(2):  # head within pair
                            p0 = sub * 64
                            psc = sc_psum.tile([P, S], F32, tag="sc")
                            for nn in range(2):
                                nc.tensor.matmul(
                                    psc[:, nn * 512 : (nn + 1) * 512],
                                    qT_sb[p0 : p0 + 64, hp, i * P : (i + 1) * P],
                                    kT_sb[p0 : p0 + 64, hp, nn * 512 : (nn + 1) * 512],
                                    start=True, stop=True,
                                )
                            ps.append(psc)
                        # rowwise min/max -> alpha/beta for fused sigmoid
                        # layout: [maxA, maxB, minA, minB, betaA, betaB, alpA, alpB]
                        st = sstat.tile([P, 8], F32, tag="st")
                        wraws = []
                        for sub in range(2):
                            wraw = wraw_pool.tile([P, S], F32, tag="wr")
                            # fused PSUM->SBUF copy + row-max (accum) in one op
                            nc.vector.tensor_scalar(
                                out=wraw[:], in0=ps[sub][:],
                                scalar1=-3.0e38, scalar2=None,
                                op0=mybir.AluOpType.max, op1=mybir.AluOpType.max,
                                accum_out=st[:, sub : sub + 1],
                            )
                            wraws.append(wraw)
                            nc.vector.tensor_reduce(
                                st[:, 2 + sub : 3 + sub], wraw[:],
                                axis=mybir.AxisListType.X, op=mybir.AluOpType.min,
                            )
                        ps = wraws
                        mx = st[:, 0:2]
                        mn = st[:, 2:4]
                        rng = st[:, 4:6]
                        alp = st[:, 6:8]
                        nc.vector.tensor_sub(rng, mx, mn)
                        nc.vector.reciprocal(rng, rng)
                        nc.vector.tensor_scalar_mul(alp, rng, 10.0)
                        # beta = -(mn*alpha) - 5  (reuse rng slot)
                        nc.vector.tensor_mul(rng, mn, alp)
                        nc.vector.tensor_scalar(
                            out=rng, in0=rng, scalar1=-1.0, scalar2=-5.0,
                            op0=mybir.AluOpType.mult, op1=mybir.AluOpType.add,
                        )
                        wts = []
                        for sub in range(2):
                            w_t = w_pool.tile([P, S], BF, tag="w")
                            nc.scalar.activation(
                                out=w_t[:], in_=ps[sub][:],
                                func=mybir.ActivationFunctionType.Sigmoid,
                                bias=st[:, 4 + sub : 5 + sub],
                                scale=st[:, 6 + sub : 7 + sub],
                            )
                            wt_t = wt_pool.tile([P, NCH, P], BF, tag="wt")
                            nc.sync.dma_start_transpose(wt_t[:], w_t[:])
                            if debug and hp == 0 and i == 0 and sub == 0:
                                nc.sync.dma_start(dbg_w_d[:], wt_t[:])
                                nc.sync.dma_start(dbg_wp_d[:], w_t[:])
                                dbg_sc_t = w_pool.tile([P, S], F32, tag="dbgsc")
                                nc.vector.tensor_copy(dbg_sc_t[:], ps[sub][:])
                                nc.sync.dma_start(dbg_sc_d[:], dbg_sc_t[:])
                                nc.sync.dma_start(dbg_st_d[:], st[:])
                            wts.append(wt_t)
                        pending.append((hp, i, wts))
                        if len(pending) > 1:
                            out_stage(pending.pop(0))
                for pp_ in pending:
                    out_stage(pp_)

    nc.compile()
    _BUILD_CACHE[(use_beta,)] = nc
    return nc


def _prep_inputs(x, emb_table, gamma, beta, Wq, Wk, Wv, use_beta):
    x = np.asarray(x)
    gamma = np.asarray(gamma, dtype=np.float32)
    beta = np.asarray(beta, dtype=np.float32)
    Wq = np.asarray(Wq, dtype=np.float32)
    Wk = np.asarray(Wk, dtype=np.float32)
    Wv = np.asarray(Wv, dtype=np.float32)
    emb = np.ascontiguousarray(np.asarray(emb_table, dtype=np.float32))

    # W'[h,a,e] = W[h,a,e] * gamma[e]; layouts [e, h*ad+a]
    wqt = np.ascontiguousarray(
        (Wq * gamma[None, None, :]).reshape(H * A, E).T.astype(BF16)
    )
    wkt = np.ascontiguousarray(
        (Wk * gamma[None, None, :]).reshape(H * A, E).T.astype(BF16)
    )
    wvt = np.ascontiguousarray(
        (Wv * gamma[None, None, :]).reshape(H * E, E).T.astype(BF16)
    )
    idn = np.eye(P, dtype=np.float32).astype(BF16)

    consts = dict(emb=emb, wqt=wqt, wkt=wkt, wvt=wvt, idn=idn)
    if use_beta:
        qb = (Wq.reshape(H * A, E) @ beta).astype(np.float32)   # [512]
        kb = (Wk.reshape(H * A, E) @ beta).astype(np.float32)
        vb = (Wv.reshape(H * E, E) @ beta).astype(np.float32)   # [4096]
        consts["qb"] = np.ascontiguousarray(qb.reshape(4, P).T)
        consts["kb"] = np.ascontiguousarray(kb.reshape(4, P).T)
        consts["vb"] = vb.reshape(1, H * E)

    in_maps = []
    for b in range(B):
        xi = x[b].astype(np.int64)
        idx16 = np.ascontiguousarray(
            xi.reshape(S // 16, 16).T.astype(np.int16)
        )  # [16, 64]; token j of chunk c sits at [j%16, 8c + j//16]
        idx_full = np.ascontiguousarray(np.tile(idx16, (8, 1)))  # [128, 64]
        in_maps.append(dict(idx=idx_full, **consts))
    return in_maps


def kernel(x, emb_table, gamma, beta, Wq, Wk, Wv):
    global LAST_RESULTS
    beta_arr = np.asarray(beta, dtype=np.float32)
    use_beta = bool(np.any(beta_arr != 0.0))

    nc = build_nc(use_beta)
    in_maps = _prep_inputs(x, emb_table, gamma, beta, Wq, Wk, Wv, use_beta)

    trace = os.environ.get("KERNEL_TRACE", "0") == "1"
    res = run_bass_kernel_spmd(
        nc, in_maps, core_ids=list(range(B)), trace=trace
    )
    LAST_RESULTS = res

    out = np.stack([np.asarray(res.results[b]["out"]) for b in range(B)], axis=0)
    return out.astype(np.float32)


if __name__ == "__main__":
    rng = np.random.default_rng(0)
    x = rng.integers(0, VOCAB, size=(B, S), dtype=np.int32)
    emb = rng.standard_normal((VOCAB, E), dtype=np.float32)
    gamma = np.ones(E, np.float32)
    beta = np.zeros(E, np.float32)
    Wq = rng.random((H, A, E), dtype=np.float32)
    Wk = rng.random((H, A, E), dtype=np.float32)
    Wv = rng.random((H, E, E), dtype=np.float32)
    out = kernel(x, emb, gamma, beta, Wq, Wk, Wv)
    print(out.shape, out.dtype)


# revision 20
# speedup vs baseline: 1.3394x; 1.1967x over previous
"""Trainium2 Bass kernel for EruSelfAttentionModel.

Math (reference, simplified):
  e  = emb_table[x]                                  # [B,S,E] gather
  h  = LayerNorm(e) * gamma + beta                   # over E
  q  = einsum('hae,bse->bhsa', Wq, h); k likewise    # A=64 per head
  v  = einsum('hve,bse->bhsv', Wv, h)                # v-dim = E
  scores = q @ k^T / sqrt(E)
  sn = (scores - min) / (max - min)  (rowwise)
  softmax_sel = 1 - max(sn) == 0 exactly  =>  weights = sigmoid(10*sn - 5)
  out = weights @ v                                  # [B,H,S,E]

Key identities used:
  - sn is invariant to positive rescaling of scores => the 1/sqrt(E) scale
    can be dropped entirely.
  - weights = sigmoid(alpha * scores + beta_row) with per-row
    alpha = 10/(mx-mn), beta_row = -10*mn/(mx-mn) - 5  => single fused
    ScalarE activation pass (per-partition scale/bias APs).

Sharding: data-parallel over batch; core b computes batch b fully.
"""

import os
import sys

sys.path.insert(0, "/opt/trn_rl_repo")

import numpy as np
import ml_dtypes

import concourse.bass as bass
import concourse.bacc as bacc
import concourse.tile as tile
from concourse import mybir
from concourse.bass_utils import run_bass_kernel_spmd
import concourse.bass_utils as _bass_utils

# Let walrus overlap LDWEIGHTS with in-flight matmuls (the PE pulls LDW for
# the background weight buffer ahead); without it every matmul serializes
# behind its weight load (~+160ns per matmul).
if not getattr(_bass_utils, "_ldw_patched", False):
    _orig_run_command = _bass_utils.run_command

    def _patched_run_command(argv, **kwargs):
        if os.environ.get("KERNEL_LDW_OPT", "1") == "1":
            argv = [
                a.replace("--enable-ldw-opt=false", "--enable-ldw-opt=true")
                if isinstance(a, str) else a
                for a in argv
            ]
        return _orig_run_command(argv, **kwargs)

    _bass_utils.run_command = _patched_run_command
    _bass_utils._ldw_patched = True

BF16 = ml_dtypes.bfloat16

VOCAB, E, A, H = 32000, 512, 64, 8
B, S = 8, 1024
P = 128                 # partitions
NCH = S // P            # 8 token chunks
EC = E // P             # 4 embedding chunks
LN_EPS = 1e-5

F32 = mybir.dt.float32
BF = mybir.dt.bfloat16
I16 = mybir.dt.int16

_BUILD_CACHE = {}
LAST_RESULTS = None     # test.py reads exec_time_ns from here


def build_nc(use_beta: bool, debug: bool = False):
    if (use_beta, debug) in _BUILD_CACHE:
        return _BUILD_CACHE[(use_beta, debug)]

    nc = bacc.Bacc("TRN2", target_bir_lowering=False, num_devices=8)

    idx_d = nc.declare_dram_parameter("idx", [P, S // 16], I16, isOutput=False)
    emb_d = nc.declare_dram_parameter("emb", [VOCAB, E], F32, isOutput=False)
    wqt_d = nc.declare_dram_parameter("wqt", [E, H * A], BF, isOutput=False)
    wkt_d = nc.declare_dram_parameter("wkt", [E, H * A], BF, isOutput=False)
    wvt_d = nc.declare_dram_parameter("wvt", [E, H * E], BF, isOutput=False)
    idn_d = nc.declare_dram_parameter("idn", [P, P], BF, isOutput=False)
    if use_beta:
        qb_d = nc.declare_dram_parameter("qb", [P, 4], F32, isOutput=False)
        kb_d = nc.declare_dram_parameter("kb", [P, 4], F32, isOutput=False)
        vb_d = nc.declare_dram_parameter("vb", [1, H * E], F32, isOutput=False)
    out_d = nc.declare_dram_parameter("out", [H, S, E], BF, isOutput=True)
    if debug:
        dbg_ht_d = nc.declare_dram_parameter("dbg_ht", [P, EC, S], BF, isOutput=True)
        dbg_qt_d = nc.declare_dram_parameter("dbg_qt", [P, EC, S], BF, isOutput=True)
        dbg_kt_d = nc.declare_dram_parameter("dbg_kt", [P, EC, S], BF, isOutput=True)
        dbg_vh_d = nc.declare_dram_parameter("dbg_vh", [P, NCH, H * E], BF, isOutput=True)
        dbg_w_d = nc.declare_dram_parameter("dbg_w", [P, NCH, P], BF, isOutput=True)
        dbg_wp_d = nc.declare_dram_parameter("dbg_wp", [P, S], BF, isOutput=True)
        dbg_sc_d = nc.declare_dram_parameter("dbg_sc", [P, S], F32, isOutput=True)
        dbg_st_d = nc.declare_dram_parameter("dbg_st", [P, 8], F32, isOutput=True)

    with tile.TileContext(nc) as tc:
        with tc.tile_pool(name="consts", bufs=1) as consts:
            idx_sb = consts.tile([P, S // 16], I16)
            nc.sync.dma_start(idx_sb[:], idx_d[:])
            idn_sb = consts.tile([P, P], BF)
            nc.sync.dma_start(idn_sb[:], idn_d[:])
            wqt_sb = consts.tile([P, EC, H * A], BF)
            nc.sync.dma_start(
                wqt_sb[:], wqt_d.ap().rearrange("(ec p) j -> p ec j", p=P)
            )
            wkt_sb = consts.tile([P, EC, H * A], BF)
            nc.sync.dma_start(
                wkt_sb[:], wkt_d.ap().rearrange("(ec p) j -> p ec j", p=P)
            )
            wvt_sb = consts.tile([P, EC, H * E], BF)
            nc.sync.dma_start(
                wvt_sb[:], wvt_d.ap().rearrange("(ec p) j -> p ec j", p=P)
            )
            eps_sb = consts.tile([P, 1], F32)
            nc.vector.memset(eps_sb[:], LN_EPS)

            if use_beta:
                qb_sb = consts.tile([P, 4], F32)
                nc.sync.dma_start(qb_sb[:], qb_d[:])
                kb_sb = consts.tile([P, 4], F32)
                nc.sync.dma_start(kb_sb[:], kb_d[:])
                vb_sb = consts.tile([P, H * E], F32)
                vb_bcast = bass.AP(
                    tensor=vb_d, offset=0, ap=[[0, P], [1, H * E]]
                )
                nc.sync.dma_start(vb_sb[:], vb_bcast)

            # persistent activations
            hT_sb = consts.tile([P, EC, S], BF)       # hT[e%128, e//128, s]
            qT_sb = consts.tile([P, EC, S], BF)       # qT[ha%128, ha//128, s]
            kT_sb = consts.tile([P, EC, S], BF)
            vh_sb = consts.tile([P, NCH, H * E], BF)  # vh[p, c, v] = V[8p+c, v]

            # ---------------- phase A: gather + LN + transpose ----------------
            with (
                tc.tile_pool(name="e_pool", bufs=1) as e_pool,
                tc.tile_pool(name="h_pool", bufs=3) as h_pool,
                tc.tile_pool(name="st_pool", bufs=8) as st_pool,
                tc.tile_pool(name="pt_psum", bufs=4, space="PSUM") as pt_psum,
            ):
                e_ts = []
                mv = st_pool.tile([P, NCH, 2], F32, tag="mv")
                for c in range(NCH):
                    e_t = e_pool.tile([P, 1, E], F32, tag=f"e{c}")
                    nc.gpsimd.dma_gather(
                        e_t[:], emb_d.ap(), idx_sb[:, 8 * c : 8 * (c + 1)],
                        P, P, E,
                    )
                    e_ts.append(e_t)
                    stt = st_pool.tile([P, 6], F32, tag="bn")
                    nc.vector.bn_stats(stt[:], e_t[:, 0, :])
                    nc.vector.bn_aggr(mv[:, c, :], stt[:])
                # batched inv-std: var <- 1/sqrt(var+eps) for all 8 chunks
                var_ap = mv[:, :, 1]
                nc.scalar.activation(
                    out=var_ap, in_=var_ap,
                    func=mybir.ActivationFunctionType.Sqrt,
                    bias=eps_sb[:, 0:1], scale=1.0,
                )
                nc.vector.reciprocal(var_ap, var_ap)
                for c in range(NCH):
                    h_t = h_pool.tile([P, E], BF)
                    nc.vector.tensor_scalar(
                        out=h_t[:], in0=e_ts[c][:, 0, :],
                        scalar1=mv[:, c, 0:1], scalar2=mv[:, c, 1:2],
                        op0=mybir.AluOpType.subtract, op1=mybir.AluOpType.mult,
                    )
                    for ec in range(EC):
                        pt = pt_psum.tile([P, P], BF)
                        nc.tensor.transpose(
                            pt[:], h_t[:, ec * P : (ec + 1) * P], idn_sb[:]
                        )
                        nc.any.tensor_copy(
                            hT_sb[:, ec, c * P : (c + 1) * P], pt[:]
                        )

            # ---------------- phase B: projections ----------------
            with tc.tile_pool(name="proj_psum", bufs=2, space="PSUM") as proj_psum:
                for name, w_sb, t_sb, b_sb in (
                    ("q", wqt_sb, qT_sb, "qb"),
                    ("k", wkt_sb, kT_sb, "kb"),
                ):
                    for sl in range(4):  # 128-wide (h,a) slices = head pairs
                        pq = proj_psum.tile([P, S], F32, tag="pp")
                        for ec in range(EC):
                            lhsT = w_sb[:, ec, sl * P : (sl + 1) * P]
                            for nn in range(2):
                                nc.tensor.matmul(
                                    pq[:, nn * 512 : (nn + 1) * 512],
                                    lhsT,
                                    hT_sb[:, ec, nn * 512 : (nn + 1) * 512],
                                    start=(ec == 0), stop=(ec == EC - 1),
                                )
                        if use_beta:
                            bb = qb_sb if name == "q" else kb_sb
                            nc.vector.tensor_scalar_add(
                                out=pq[:], in0=pq[:], scalar1=bb[:, sl : sl + 1]
                            )
                        nc.any.tensor_copy(t_sb[:, sl, :], pq[:])

            if debug:
                nc.sync.dma_start(dbg_ht_d[:], hT_sb[:])
                nc.sync.dma_start(dbg_qt_d[:], qT_sb[:])
                nc.sync.dma_start(dbg_kt_d[:], kT_sb[:])
                nc.sync.dma_start(dbg_vh_d[:], vh_sb[:])

            # ---------------- phase C: attention ----------------
            with (
                tc.tile_pool(name="sc_psum", bufs=3, space="PSUM") as sc_psum,
                tc.tile_pool(name="out_psum", bufs=2, space="PSUM") as out_psum,
                tc.tile_pool(name="sstat", bufs=8) as sstat,
                tc.tile_pool(name="w_pool", bufs=6) as w_pool,
                tc.tile_pool(name="wraw_pool", bufs=5) as wraw_pool,
                tc.tile_pool(name="wt_pool", bufs=18) as wt_pool,
                tc.tile_pool(name="ob_pool", bufs=3) as ob_pool,
            ):
                def vhat_group(c):
                    # V-hat projection for t-group c, interleaved into phase C
                    # to keep TensorE dense (HAM stays un-throttled).
                    for vp in range(4):
                        pv = sc_psum.tile([P, S], F32, tag="sc")
                        for ec in range(EC):
                            lhsT = hT_sb[:, ec, c * P : (c + 1) * P]
                            for nn in range(2):
                                lo = vp * 1024 + nn * 512
                                nc.tensor.matmul(
                                    pv[:, nn * 512 : (nn + 1) * 512],
                                    lhsT,
                                    wvt_sb[:, ec, lo : lo + 512],
                                    start=(ec == 0), stop=(ec == EC - 1),
                                )
                        nc.any.tensor_copy(
                            vh_sb[:, c, vp * 1024 : (vp + 1) * 1024], pv[:]
                        )

                def out_stage(args):
                    hp_, i_, wts = args
                    for sub in range(2):
                        h_idx = 2 * hp_ + sub
                        wt_t = wts[sub]
                        po = out_psum.tile([P, E], F32, tag="po")
                        for cc in range(NCH):
                            nc.tensor.matmul(
                                po[:],
                                wt_t[:, cc, :],
                                vh_sb[:, cc, h_idx * E : (h_idx + 1) * E],
                                start=(cc == 0), stop=(cc == NCH - 1),
                            )
                        ob = ob_pool.tile([P, E], BF, tag="ob")
                        nc.scalar.copy(ob[:], po[:])
                        nc.sync.dma_start(
                            out_d[h_idx, i_ * P : (i_ + 1) * P, :], ob[:]
                        )

                pending = []
                unit_no = 0
                for hp in range(4):      # head pair (2hp, 2hp+1)
                    for i in range(NCH):  # query chunk
                        if unit_no < NCH:
                            vhat_group(unit_no)
                        unit_no += 1
                        ps = []
                        for sub in range(2):  # head within pair
                            p0 = sub * 64
                            psc = sc_psum.tile([P, S], F32, tag="sc")
                            for nn in range(2):
                                nc.tensor.matmul(
                                    psc[:, nn * 512 : (nn + 1) * 512],
                                    qT_sb[p0 : p0 + 64, hp, i * P : (i + 1) * P],
                                    kT_sb[p0 : p0 + 64, hp, nn * 512 : (nn + 1) * 512],
                                    start=True, stop=True,
                                )
                            ps.append(psc)
                        # rowwise min/max -> alpha/beta for fused sigmoid
                        # layout: [maxA, maxB, minA, minB, betaA, betaB, alpA, alpB]
                        st = sstat.tile([P, 8], F32, tag="st")
                        wraws = []
                        for sub in range(2):
                            wraw = wraw_pool.tile([P, S], F32, tag="wr")
                            # fused PSUM->SBUF copy + row-max (accum) in one op
                            nc.vector.tensor_scalar(
                                out=wraw[:], in0=ps[sub][:],
                                scalar1=-3.0e38, scalar2=None,
                                op0=mybir.AluOpType.max, op1=mybir.AluOpType.max,
                                accum_out=st[:, sub : sub + 1],
                            )
                            wraws.append(wraw)
                            nc.vector.tensor_reduce(
                                st[:, 2 + sub : 3 + sub], wraw[:],
                                axis=mybir.AxisListType.X, op=mybir.AluOpType.min,
                            )
                        ps = wraws
                        mx = st[:, 0:2]
                        mn = st[:, 2:4]
                        rng = st[:, 4:6]
                        alp = st[:, 6:8]
                        nc.vector.tensor_sub(rng, mx, mn)
                        nc.vector.reciprocal(rng, rng)
                        nc.vector.tensor_scalar_mul(alp, rng, 10.0)
                        # beta = -(mn*alpha) - 5  (reuse rng slot)
                        nc.vector.tensor_mul(rng, mn, alp)
                        nc.vector.tensor_scalar(
                            out=rng, in0=rng, scalar1=-1.0, scalar2=-5.0,
                            op0=mybir.AluOpType.mult, op1=mybir.AluOpType.add,
                        )
                        wts = []
                        for sub in range(2):
                            w_t = w_pool.tile([P, S], BF, tag="w")
                            nc.scalar.activation(
                                out=w_t[:], in_=ps[sub][:],
                                func=mybir.ActivationFunctionType.Sigmoid,
                                bias=st[:, 4 + sub : 5 + sub],
                                scale=st[:, 6 + sub : 7 + sub],
                            )
                            wt_t = wt_pool.tile([P, NCH, P], BF, tag="wt")
                            nc.sync.dma_start_transpose(wt_t[:], w_t[:])
                            if debug and hp == 0 and i == 0 and sub == 0:
                                nc.sync.dma_start(dbg_w_d[:], wt_t[:])
                                nc.sync.dma_start(dbg_wp_d[:], w_t[:])
                                dbg_sc_t = w_pool.tile([P, S], F32, tag="dbgsc")
                                nc.vector.tensor_copy(dbg_sc_t[:], ps[sub][:])
                                nc.sync.dma_start(dbg_sc_d[:], dbg_sc_t[:])
                                nc.sync.dma_start(dbg_st_d[:], st[:])
                            wts.append(wt_t)
                        pending.append((hp, i, wts))
                        if len(pending) > 8:
                            out_stage(pending.pop(0))
                for pp_ in pending:
                    out_stage(pp_)

    nc.compile()
    _BUILD_CACHE[(use_beta,)] = nc
    return nc


def _prep_inputs(x, emb_table, gamma, beta, Wq, Wk, Wv, use_beta):
    x = np.asarray(x)
    gamma = np.asarray(gamma, dtype=np.float32)
    beta = np.asarray(beta, dtype=np.float32)
    Wq = np.asarray(Wq, dtype=np.float32)
    Wk = np.asarray(Wk, dtype=np.float32)
    Wv = np.asarray(Wv, dtype=np.float32)
    emb = np.ascontiguousarray(np.asarray(emb_table, dtype=np.float32))

    # W'[h,a,e] = W[h,a,e] * gamma[e]; layouts [e, h*ad+a]
    wqt = np.ascontiguousarray(
        (Wq * gamma[None, None, :]).reshape(H * A, E).T.astype(BF16)
    )
    wkt = np.ascontiguousarray(
        (Wk * gamma[None, None, :]).reshape(H * A, E).T.astype(BF16)
    )
    wvt = np.ascontiguousarray(
        (Wv * gamma[None, None, :]).reshape(H * E, E).T.astype(BF16)
    )
    idn = np.eye(P, dtype=np.float32).astype(BF16)

    consts = dict(emb=emb, wqt=wqt, wkt=wkt, wvt=wvt, idn=idn)
    if use_beta:
        qb = (Wq.reshape(H * A, E) @ beta).astype(np.float32)   # [512]
        kb = (Wk.reshape(H * A, E) @ beta).astype(np.float32)
        vb = (Wv.reshape(H * E, E) @ beta).astype(np.float32)   # [4096]
        consts["qb"] = np.ascontiguousarray(qb.reshape(4, P).T)
        consts["kb"] = np.ascontiguousarray(kb.reshape(4, P).T)
        consts["vb"] = vb.reshape(1, H * E)

    in_maps = []
    for b in range(B):
        xi = x[b].astype(np.int64)
        idx16 = np.ascontiguousarray(
            xi.reshape(S // 16, 16).T.astype(np.int16)
        )  # [16, 64]; token j of chunk c sits at [j%16, 8c + j//16]
        idx_full = np.ascontiguousarray(np.tile(idx16, (8, 1)))  # [128, 64]
        in_maps.append(dict(idx=idx_full, **consts))
    return in_maps


def kernel(x, emb_table, gamma, beta, Wq, Wk, Wv):
    global LAST_RESULTS
    beta_arr = np.asarray(beta, dtype=np.float32)
    use_beta = bool(np.any(beta_arr != 0.0))

    nc = build_nc(use_beta)
    in_maps = _prep_inputs(x, emb_table, gamma, beta, Wq, Wk, Wv, use_beta)

    trace = os.environ.get("KERNEL_TRACE", "0") == "1"
    res = run_bass_kernel_spmd(
        nc, in_maps, core_ids=list(range(B)), trace=trace
    )
    LAST_RESULTS = res

    out = np.stack([np.asarray(res.results[b]["out"]) for b in range(B)], axis=0)
    return out.astype(np.float32)


if __name__ == "__main__":
    rng = np.random.default_rng(0)
    x = rng.integers(0, VOCAB, size=(B, S), dtype=np.int32)
    emb = rng.standard_normal((VOCAB, E), dtype=np.float32)
    gamma = np.ones(E, np.float32)
    beta = np.zeros(E, np.float32)
    Wq = rng.random((H, A, E), dtype=np.float32)
    Wk = rng.random((H, A, E), dtype=np.float32)
    Wv = rng.random((H, E, E), dtype=np.float32)
    out = kernel(x, emb, gamma, beta, Wq, Wk, Wv)
    print(out.shape, out.dtype)
